# revision 20
# baseline (speedup 1.0000x reference)
"""BiDAF attention kernel for 8 Trainium2 NeuronCores.

Data-parallel over batch (B=32 -> 4 per core). Per batch, on-chip:
  sT[j,i] = (q*cqw) @ c^T + s0[i] + (s1[j]+bias)   (bf16 matmuls, fp32 accum)
  s0 = c @ c_weight and s1b = q @ q_weight + bias are host-precomputed
  (tiny rank-1 terms); s0 enters via a K=1 f32r matmul accumulated into the
  same PSUM bank, s1b via the exp activation's per-partition bias.
  E = exp(sT)  (one exp serves both softmaxes; rowsum via accum_out)
  a1 normalization deferred: rS=1/colsum(E) scales the a/b PSUM evacuations;
  a2 normalization deferred: ra2=1/rowsum(E) folds into the M2 evacuation.
  a = a1 @ q; b = a1 @ (a2^T @ c); device stores [a, b] in bf16.
Key perf structure vs the previous version:
  - c is never cast on an engine: PE transposes c as f32r (1.5 cy/row) and
    the PSUM->SBUF evacuation casts to bf16 for the sT matmul rhs.
  - q is cast f32->bf16 in flight by the gpsimd software-DGE DMA.
  - M2 = a2^T @ c runs as an f32r matmul straight off the f32 c tile.
  - c*a and c*b are computed on the host during unshard (the device writes
    only [a, b]: 4 MiB of bf16 HBM writes per core instead of 6+).
  - identity + small consts issue ahead of the bulk loads; one batched 1 MiB
    store per batch; stages interleaved so the PE pipeline never drains
    (p-state ramp: a continuously-busy PE runs 2x faster than one with gaps).
"""

import sys

if "/opt/trn_rl_repo" not in sys.path:
    sys.path.insert(0, "/opt/trn_rl_repo")

from contextlib import ExitStack

import numpy as np

import concourse.bacc as bacc
import concourse.bass as bass
import concourse.mybir as mybir
from concourse.bass import ts
from concourse.bass_utils import run_bass_kernel_spmd
from concourse.masks import make_identity
from concourse.tile import TileContext

N_CORES = 8
B, Lc, Lq, H = 32, 512, 64, 512
BPC = B // N_CORES  # batches per core
F32 = mybir.dt.float32
F32R = mybir.dt.float32r
BF16 = mybir.dt.bfloat16

_CACHE = {}


def _build_program():
    nc = bacc.Bacc("TRN2", target_bir_lowering=False, debug=False, num_devices=N_CORES)
    c_h = nc.dram_tensor("c", [BPC, Lc, H], F32R, kind="ExternalInput")
    q_h = nc.dram_tensor("q", [BPC, Lq, H], F32R, kind="ExternalInput")
    cqw_h = nc.dram_tensor("cqw", [H], F32, kind="ExternalInput")
    s0_h = nc.dram_tensor("s0", [BPC, Lc], F32R, kind="ExternalInput")
    s1b_h = nc.dram_tensor("s1b", [BPC, Lq], F32, kind="ExternalInput")
    out_h = nc.dram_tensor("out", [BPC, 128, 4, 2, H], BF16, kind="ExternalOutput")

    c_ap = c_h.ap()
    q_ap = q_h.ap()
    out_ap = out_h.ap()

    exp_f = mybir.ActivationFunctionType.Exp
    copy_f = mybir.ActivationFunctionType.Copy

    with TileContext(nc) as tc, ExitStack() as ctx:
        const = ctx.enter_context(tc.tile_pool(name="const", bufs=1))
        cpool = ctx.enter_context(tc.tile_pool(name="cpool", bufs=4))
        ctpool = ctx.enter_context(tc.tile_pool(name="ctpool", bufs=2))
        lhpool = ctx.enter_context(tc.tile_pool(name="lhpool", bufs=2))
        qpool = ctx.enter_context(tc.tile_pool(name="qpool", bufs=4))
        qbpool = ctx.enter_context(tc.tile_pool(name="qbpool", bufs=4))
        spool = ctx.enter_context(tc.tile_pool(name="spool", bufs=12))
        epool = ctx.enter_context(tc.tile_pool(name="epool", bufs=4))
        btpool = ctx.enter_context(tc.tile_pool(name="btpool", bufs=2))
        mpool = ctx.enter_context(tc.tile_pool(name="mpool", bufs=3))
        opool = ctx.enter_context(tc.tile_pool(name="opool", bufs=2))
        ps_tr = ctx.enter_context(tc.tile_pool(name="ps_tr", bufs=2, space="PSUM"))
        ps_trq = ctx.enter_context(tc.tile_pool(name="ps_trq", bufs=1, space="PSUM"))
        ps_mm = ctx.enter_context(tc.tile_pool(name="ps_mm", bufs=2, space="PSUM"))
        ps_sm = ctx.enter_context(tc.tile_pool(name="ps_sm", bufs=1, space="PSUM"))
        ps_ab = ctx.enter_context(tc.tile_pool(name="ps_ab", bufs=2, space="PSUM"))

        # ---- constants + loads: identity first on the gpsimd queue so the
        # first PE transposes are never gated on it; q casts f32->bf16 in
        # flight (SWDGE); c goes f32 on the sync HWDGE queue; small consts
        # issue from the scalar HWDGE queue ahead of its activation work ----
        ident = const.tile([128, 128], BF16, name="ident")
        make_identity(nc, ident)
        identf = const.tile([128, 128], F32R, name="identf")
        nc.vector.tensor_copy(out=identf, in_=ident)

        ones_col = const.tile([Lq, 1], BF16, name="ones_col")
        nc.vector.memset(ones_col, 1.0)
        ones_f = const.tile([1, Lq], F32, name="ones_f")
        nc.vector.memset(ones_f, 1.0)
        onesK = const.tile([1, Lq], F32R, name="onesK")
        nc.vector.tensor_copy(out=onesK, in_=ones_f)

        # Load scheduling: a single DMA stream tops out at ~200 GB/s and
        # concurrent DMAs split HBM bandwidth evenly, so c0 goes out as 4
        # partition-range quarters, two per HWDGE queue, issued before
        # anything else (full-bandwidth arrival of the first batch by ~10us).
        # c1..c3 are half-DMA pairs on the sync queue, each pair gated on the
        # previous batch's completion via 16-byte gpsimd copies into the next
        # tile's head bytes (the DMA waits on the WAW dep; one gate per
        # previous-writer partition range so every sub-DMA is covered).
        q_tiles = {}
        c_tiles = {}
        for bb in range(BPC):
            q_tiles[bb] = qpool.tile([Lq, H], F32R, name="q_sb")
            c_tiles[bb] = cpool.tile([128, 4, H], F32R, name="c_sb")

        def c_src(bb):
            return c_ap[bb].rearrange("(p j) h -> p j h", p=128)

        def gate(nxt, prev, parts):
            for p in parts:
                nc.gpsimd.tensor_copy(
                    out=nxt[p : p + 1, 0, 0:4], in_=prev[p : p + 1, 0, 0:4]
                )

        nc.sync.dma_start(out=c_tiles[0], in_=c_src(0))

        cqw_t = const.tile([128, 4], F32, name="cqw_t")
        nc.scalar.dma_start(
            out=cqw_t, in_=bass.AP(tensor=cqw_h, offset=0, ap=[[1, 128], [128, 4]])
        )
        s1b_t = const.tile([Lq, BPC], F32, name="s1b_t")
        nc.scalar.dma_start(
            out=s1b_t, in_=bass.AP(tensor=s1b_h, offset=0, ap=[[1, Lq], [Lq, BPC]])
        )
        s0_t = const.tile([1, BPC * Lc], F32R, name="s0_t")
        nc.scalar.dma_start(
            out=s0_t, in_=bass.AP(tensor=s0_h, offset=0, ap=[[1, 1], [1, BPC * Lc]])
        )
        nc.scalar.dma_start(out=q_tiles[0], in_=q_ap[0])
        # prime the activation table (1.3us) while the loads are in flight
        scr2 = const.tile([1, Lq], F32, name="scr2")
        nc.scalar.activation(out=scr2, in_=ones_f, func=exp_f)
        for bb in range(1, BPC):
            nc.scalar.dma_start(out=q_tiles[bb], in_=q_ap[bb])

        for bb in range(1, BPC):
            gate(c_tiles[bb], c_tiles[bb - 1], (0,))
            nc.sync.dma_start(out=c_tiles[bb], in_=c_src(bb))

        S = [dict() for _ in range(BPC)]  # per-batch tile state

        def stage_A(b):
            """c transposes (f32r) -> qs^T -> sT matmuls + s0 aug -> exp"""
            c_sb = c_tiles[b]
            q_sb = q_tiles[b]

            # cT[f] = c^T chunk (h rows f*128.., all Lc cols); evac casts->bf16
            cT = ctpool.tile([128, 4, H], BF16, name="cT")
            for j in range(4):
                pt_c = ps_tr.tile([128, 4, 128], F32R, name="pt_c", tag="tr")
                for f in range(4):
                    nc.tensor.transpose(pt_c[:, f, :], c_sb[:, j, ts(f, 128)], identf)
                if j % 2 == 0:
                    nc.vector.tensor_copy(out=cT[:, :, ts(j, 128)], in_=pt_c)
                else:
                    nc.scalar.activation(
                        out=cT[:, :, ts(j, 128)], in_=pt_c, func=copy_f
                    )

            # qs^T = (q * cqw)^T via PE transpose + per-partition cqw scale
            lhsT = lhpool.tile([128, 4, Lq], BF16, name="lhsT")
            pt_q = ps_trq.tile([128, 4, Lq], F32R, name="pt_q", tag="trq")
            for f in range(4):
                nc.tensor.transpose(pt_q[:, f, :], q_sb[:, ts(f, 128)], identf[0:Lq, 0:Lq])
            for f in range(4):
                nc.vector.tensor_scalar_mul(
                    lhsT[:, f, :], pt_q[:, f, :], cqw_t[:, f : f + 1]
                )
            # bf16 q for stage C's a-matmul rhs; Pool is idle and the result
            # is not needed until C(b), so its slow cast rate is fine
            q_bf = qbpool.tile([Lq, H], BF16, name="q_bf")
            nc.gpsimd.tensor_copy(out=q_bf, in_=q_sb)

            # sT rows 0..63 = qs @ cT; then s0 broadcast via K=1 f32r matmul
            ps_sT = ps_mm.tile([128, 512], F32, name="ps_sT", tag="big1")
            for f in range(4):
                nc.tensor.matmul(
                    ps_sT[0:Lq, :], lhsT[:, f, :], cT[:, f, :],
                    start=(f == 0), stop=False,
                )
            nc.tensor.matmul(
                ps_sT[0:Lq, :], onesK, s0_t[0:1, ts(b, Lc)],
                start=False, stop=True,
            )

            # E = exp(sT + s1b) in bf16; rowsum (f32) for a2
            E_sb = epool.tile([Lq, H], BF16, name="E_sb")
            rowsum = spool.tile([Lq, 1], F32, name="rowsum")
            nc.scalar.activation(
                out=E_sb, in_=ps_sT[0:Lq, :], func=exp_f,
                bias=s1b_t[:, b : b + 1], scale=1.0, accum_out=rowsum,
            )
            S[b].update(c_sb=c_sb, q_bf=q_bf, E_sb=E_sb, rowsum=rowsum)

        def stage_B(b):
            """normalizers -> E transpose -> M2 = a2^T @ c (f32r)"""
            c_sb = S[b]["c_sb"]
            E_sb = S[b]["E_sb"]
            ra2 = spool.tile([Lq, 1], F32, name="ra2")
            nc.vector.reciprocal(ra2, S[b]["rowsum"])

            # column sums of E (normalizer of a1), one batched reciprocal
            ps_S = ps_sm.tile([128, 4], F32, name="ps_S", tag="small")
            for m in range(4):
                nc.tensor.matmul(
                    ps_S[:, m : m + 1], E_sb[:, ts(m, 128)], ones_col,
                    start=True, stop=True,
                )
            rS = spool.tile([128, 4], F32, name="rS")
            nc.vector.reciprocal(rS, ps_S)

            # E^T chunks for M2's lhsT (f32r to match the f32 c rhs)
            pt_a = ps_trq.tile([128, 4, Lq], BF16, name="pt_a", tag="trq")
            for f in range(4):
                nc.tensor.transpose(pt_a[:, f, :], E_sb[:, ts(f, 128)], ident[0:Lq, 0:Lq])
            a2n = btpool.tile([128, 4, Lq], F32R, name="a2n")
            nc.scalar.activation(out=a2n, in_=pt_a, func=copy_f)

            # M2 = a2^T @ c  [Lq, H]; evac applies ra2, casts to bf16
            ps_M2 = ps_mm.tile([128, 512], F32, name="ps_M2", tag="big1")
            for j in range(4):
                nc.tensor.matmul(
                    ps_M2[0:Lq, :], a2n[:, j, :], c_sb[:, j, :],
                    start=(j == 0), stop=(j == 3),
                )
            M2_sb = mpool.tile([Lq, H], BF16, name="M2_sb")
            nc.scalar.activation(
                out=M2_sb, in_=ps_M2[0:Lq, :], func=copy_f, scale=ra2
            )
            S[b].update(rS=rS, M2_sb=M2_sb)

        def stage_C(b, ms=(0, 1, 2, 3)):
            """per i-tile: a = a1@q, b = a1@M2, rS-scaled bf16 evacs"""
            q_bf = S[b]["q_bf"]
            E_sb = S[b]["E_sb"]
            rS = S[b]["rS"]
            M2_sb = S[b]["M2_sb"]
            if "stage" not in S[b]:
                S[b]["stage"] = opool.tile([128, 4, 2, H], BF16, name="stage")
            stage = S[b]["stage"]
            for m in ms:
                ps_a = ps_ab.tile([128, 512], F32, name="ps_a", tag="big2")
                nc.tensor.matmul(
                    ps_a, E_sb[:, ts(m, 128)], q_bf, start=True, stop=True
                )
                # half evacs on both engines in parallel: the PSUM bank frees
                # in ~390ns instead of ~750, keeping pace with the matmuls
                nc.scalar.activation(
                    out=stage[:, m, 0, 0:256], in_=ps_a[:, 0:256], func=copy_f,
                    scale=rS[:, m : m + 1],
                )
                nc.vector.tensor_scalar_mul(
                    stage[:, m, 0, 256:512], ps_a[:, 256:512], rS[:, m : m + 1]
                )
                ps_b = ps_ab.tile([128, 512], F32, name="ps_b", tag="big2")
                nc.tensor.matmul(
                    ps_b, E_sb[:, ts(m, 128)], M2_sb, start=True, stop=True
                )
                nc.scalar.activation(
                    out=stage[:, m, 1, 0:256], in_=ps_b[:, 0:256], func=copy_f,
                    scale=rS[:, m : m + 1],
                )
                nc.vector.tensor_scalar_mul(
                    stage[:, m, 1, 256:512], ps_b[:, 256:512], rS[:, m : m + 1]
                )
            # half-batch stores start HBM writes as soon as two i-tiles are
            # done instead of waiting for the full batch
            o_view = out_ap[b]
            if ms[-1] == 1:
                nc.sync.dma_start(out=o_view[:, 0:2], in_=stage[:, 0:2])
            elif ms[-1] == 3:
                if ms[0] == 0:
                    nc.sync.dma_start(out=o_view[:, 0:2], in_=stage[:, 0:2])
                nc.sync.dma_start(out=o_view[:, 2:4], in_=stage[:, 2:4])

        # emission order: A stages early (deps land early), C split in halves
        # to interleave with B so every engine queue always has ready work
        stage_A(0)
        stage_A(1)
        stage_B(0)
        stage_A(2)
        stage_B(1)
        stage_C(0, (0, 1))
        stage_A(3)
        stage_B(2)
        stage_C(0, (2, 3))
        stage_C(1, (0, 1))
        stage_B(3)
        stage_C(1, (2, 3))
        stage_C(2, (0, 1))
        stage_C(2, (2, 3))
        stage_C(3)

    nc.compile()
    return nc


def _numpy_fallback(c, q, c_mask, q_mask, c_weight, q_weight, cq_weight, bias):
    NEG_INF = -1e30
    s0 = c @ c_weight
    s1 = (q @ q_weight).transpose(0, 2, 1)
    s2 = np.einsum("bih,bjh->bij", c * cq_weight, q)
    s = s0 + s1 + s2 + bias

    def softmax(x, mask, axis):
        logits = np.where(mask, x, NEG_INF)
        m = logits.max(axis=axis, keepdims=True)
        e = np.exp(logits - m)
        return e / e.sum(axis=axis, keepdims=True)

    a1 = softmax(s, q_mask[:, None, :], 2)
    a2 = softmax(s, c_mask[:, :, None], 1)
    a = np.einsum("bij,bjh->bih", a1, q)
    bb = np.einsum("bij,bjk->bik", np.einsum("bik,bjk->bij", a1, a2), c)
    return np.concatenate([c, a, c * a, c * bb], axis=2).astype(np.float32)


def _make_in_maps(c, q, cq_weight, c_weight, q_weight, bias):
    cqw = np.ascontiguousarray(np.asarray(cq_weight, np.float32).reshape(H))
    cwgt = np.asarray(c_weight, np.float32).reshape(H)
    qwgt = np.asarray(q_weight, np.float32).reshape(H)
    b0 = float(np.asarray(bias, np.float32).reshape(1)[0])
    s0 = (c.reshape(-1, H) @ cwgt).reshape(B, Lc).astype(np.float32)
    # device cT columns are in kappa = j*128 + p order where row i = 4p + j
    s0 = np.ascontiguousarray(
        s0.reshape(B, 128, 4).transpose(0, 2, 1).reshape(B, Lc)
    )
    s1b = ((q.reshape(-1, H) @ qwgt).reshape(B, Lq) + b0).astype(np.float32)
    in_maps = []
    for k in range(N_CORES):
        sl = slice(k * BPC, (k + 1) * BPC)
        in_maps.append(
            {
                "c": np.ascontiguousarray(c[sl]),
                "q": np.ascontiguousarray(q[sl]),
                "cqw": cqw,
                "s0": np.ascontiguousarray(s0[sl]),
                "s1b": np.ascontiguousarray(s1b[sl]),
            }
        )
    return in_maps


def _assemble(c, results):
    out = np.empty((B, Lc, 4 * H), dtype=np.float32)
    out[:, :, 0:H] = c
    for k in range(N_CORES):
        sl = slice(k * BPC, (k + 1) * BPC)
        ab = results[k]["out"].reshape(BPC, Lc, 2, H).astype(np.float32)
        a = ab[:, :, 0, :]
        bb = ab[:, :, 1, :]
        ck = c[sl]
        out[sl, :, H : 2 * H] = a
        np.multiply(ck, a, out=out[sl, :, 2 * H : 3 * H])
        np.multiply(ck, bb, out=out[sl, :, 3 * H : 4 * H])
    return out


def kernel(c, q, c_mask, q_mask, c_weight, q_weight, cq_weight, bias, **_):
    c = np.asarray(c, dtype=np.float32)
    q = np.asarray(q, dtype=np.float32)
    if not (np.all(c_mask) and np.all(q_mask)):
        # masks are all-ones per the problem spec; keep a correct fallback
        return _numpy_fallback(
            c, q, np.asarray(c_mask), np.asarray(q_mask),
            np.asarray(c_weight, np.float32), np.asarray(q_weight, np.float32),
            np.asarray(cq_weight, np.float32), np.asarray(bias, np.float32),
        )

    if "nc" not in _CACHE:
        _CACHE["nc"] = _build_program()
    nc = _CACHE["nc"]

    in_maps = _make_in_maps(c, q, cq_weight, c_weight, q_weight, bias)
    res = run_bass_kernel_spmd(nc, in_maps, core_ids=list(range(N_CORES)))
    return _assemble(c, res.results)


# revision 22
# speedup vs baseline: 1.1829x; 1.1829x over previous
"""BiDAF attention kernel for 8 Trainium2 NeuronCores.

Data-parallel over batch (B=32 -> 4 per core). Per batch, on-chip:
  sT[j,i] = (q*cqw) @ c^T + s0[i] + (s1[j]+bias)   (bf16 matmuls, fp32 accum)
  s0 = c @ c_weight and s1b = q @ q_weight + bias are host-precomputed
  (tiny rank-1 terms); s0 enters via a K=1 f32r matmul accumulated into the
  same PSUM bank, s1b via the exp activation's per-partition bias.
  E = exp(sT)  (one exp serves both softmaxes; rowsum via accum_out)
  a1 normalization deferred: rS=1/colsum(E) scales the a/b PSUM evacuations;
  a2 normalization deferred: ra2=1/rowsum(E) folds into the M2 evacuation.
  a = a1 @ q; b = a1 @ (a2^T @ c); device stores [a, b] in bf16.
Key perf structure vs the previous version:
  - c is never cast on an engine: PE transposes c as f32r (1.5 cy/row) and
    the PSUM->SBUF evacuation casts to bf16 for the sT matmul rhs.
  - q is cast f32->bf16 in flight by the gpsimd software-DGE DMA.
  - M2 = a2^T @ c runs as an f32r matmul straight off the f32 c tile.
  - c*a and c*b are computed on the host during unshard (the device writes
    only [a, b]: 4 MiB of bf16 HBM writes per core instead of 6+).
  - identity + small consts issue ahead of the bulk loads; one batched 1 MiB
    store per batch; stages interleaved so the PE pipeline never drains
    (p-state ramp: a continuously-busy PE runs 2x faster than one with gaps).
"""

import sys

if "/opt/trn_rl_repo" not in sys.path:
    sys.path.insert(0, "/opt/trn_rl_repo")

from contextlib import ExitStack

import numpy as np

import concourse.bacc as bacc
import concourse.bass as bass
import concourse.mybir as mybir
from concourse.bass import ts
from concourse.bass_utils import run_bass_kernel_spmd
from concourse.masks import make_identity
from concourse.tile import TileContext

N_CORES = 8
B, Lc, Lq, H = 32, 512, 64, 512
BPC = B // N_CORES  # batches per core
F32 = mybir.dt.float32
F32R = mybir.dt.float32r
BF16 = mybir.dt.bfloat16

_CACHE = {}


def _build_program():
    nc = bacc.Bacc("TRN2", target_bir_lowering=False, debug=False, num_devices=N_CORES)
    c_h = nc.dram_tensor("c", [BPC, Lc, H], F32R, kind="ExternalInput")
    q_h = nc.dram_tensor("q", [BPC, Lq, H], F32R, kind="ExternalInput")
    cqw_h = nc.dram_tensor("cqw", [H], F32, kind="ExternalInput")
    s0_h = nc.dram_tensor("s0", [BPC, Lc], F32R, kind="ExternalInput")
    s1b_h = nc.dram_tensor("s1b", [BPC, Lq], F32, kind="ExternalInput")
    out_h = nc.dram_tensor("out", [BPC, 128, 4, 2, H], BF16, kind="ExternalOutput")

    c_ap = c_h.ap()
    q_ap = q_h.ap()
    out_ap = out_h.ap()

    exp_f = mybir.ActivationFunctionType.Exp
    copy_f = mybir.ActivationFunctionType.Copy

    with TileContext(nc) as tc, ExitStack() as ctx:
        const = ctx.enter_context(tc.tile_pool(name="const", bufs=1))
        cpool = ctx.enter_context(tc.tile_pool(name="cpool", bufs=4))
        ctpool = ctx.enter_context(tc.tile_pool(name="ctpool", bufs=2))
        cbfpool = ctx.enter_context(tc.tile_pool(name="cbfpool", bufs=4))
        lhpool = ctx.enter_context(tc.tile_pool(name="lhpool", bufs=2))
        qpool = ctx.enter_context(tc.tile_pool(name="qpool", bufs=4))
        qbpool = ctx.enter_context(tc.tile_pool(name="qbpool", bufs=4))
        spool = ctx.enter_context(tc.tile_pool(name="spool", bufs=12))
        epool = ctx.enter_context(tc.tile_pool(name="epool", bufs=4))
        btpool = ctx.enter_context(tc.tile_pool(name="btpool", bufs=2))
        mpool = ctx.enter_context(tc.tile_pool(name="mpool", bufs=3))
        opool = ctx.enter_context(tc.tile_pool(name="opool", bufs=2))
        ps_tr = ctx.enter_context(tc.tile_pool(name="ps_tr", bufs=2, space="PSUM"))
        ps_trq = ctx.enter_context(tc.tile_pool(name="ps_trq", bufs=1, space="PSUM"))
        ps_mm = ctx.enter_context(tc.tile_pool(name="ps_mm", bufs=2, space="PSUM"))
        ps_ab = ctx.enter_context(tc.tile_pool(name="ps_ab", bufs=3, space="PSUM"))

        # ---- constants + loads: identity first on the gpsimd queue so the
        # first PE transposes are never gated on it; q casts f32->bf16 in
        # flight (SWDGE); c goes f32 on the sync HWDGE queue; small consts
        # issue from the scalar HWDGE queue ahead of its activation work ----
        ident = const.tile([128, 128], BF16, name="ident")
        make_identity(nc, ident)
        identf = const.tile([128, 128], F32R, name="identf")
        nc.vector.tensor_copy(out=identf, in_=ident)

        ones_col = const.tile([Lq, 1], BF16, name="ones_col")
        nc.vector.memset(ones_col, 1.0)
        ones_f = const.tile([1, Lq], F32, name="ones_f")
        nc.vector.memset(ones_f, 1.0)
        onesK = const.tile([1, Lq], F32R, name="onesK")
        nc.vector.tensor_copy(out=onesK, in_=ones_f)

        # Load scheduling: a single DMA stream tops out at ~200 GB/s and
        # concurrent DMAs split HBM bandwidth evenly, so c0 goes out as 4
        # partition-range quarters, two per HWDGE queue, issued before
        # anything else (full-bandwidth arrival of the first batch by ~10us).
        # c1..c3 are half-DMA pairs on the sync queue, each pair gated on the
        # previous batch's completion via 16-byte gpsimd copies into the next
        # tile's head bytes (the DMA waits on the WAW dep; one gate per
        # previous-writer partition range so every sub-DMA is covered).
        q_tiles = {}
        c_tiles = {}
        for bb in range(BPC):
            q_tiles[bb] = qpool.tile([Lq, H], F32R, name="q_sb")
            c_tiles[bb] = cpool.tile([128, 4, H], F32R, name="c_sb")

        def c_src(bb):
            return c_ap[bb].rearrange("(p j) h -> p j h", p=128)

        def gate(nxt, prev, parts):
            for p in parts:
                nc.gpsimd.tensor_copy(
                    out=nxt[p : p + 1, 0, 0:4], in_=prev[p : p + 1, 0, 0:4]
                )

        nc.sync.dma_start(out=c_tiles[0], in_=c_src(0))

        cqw_t = const.tile([128, 4], F32, name="cqw_t")
        nc.scalar.dma_start(
            out=cqw_t, in_=bass.AP(tensor=cqw_h, offset=0, ap=[[1, 128], [128, 4]])
        )
        s1b_t = const.tile([Lq, BPC], F32, name="s1b_t")
        nc.scalar.dma_start(
            out=s1b_t, in_=bass.AP(tensor=s1b_h, offset=0, ap=[[1, Lq], [Lq, BPC]])
        )
        s0_t = const.tile([1, BPC * Lc], F32R, name="s0_t")
        nc.scalar.dma_start(
            out=s0_t, in_=bass.AP(tensor=s0_h, offset=0, ap=[[1, 1], [1, BPC * Lc]])
        )
        nc.scalar.dma_start(out=q_tiles[0], in_=q_ap[0])
        # prime the activation table (1.3us) while the loads are in flight
        scr2 = const.tile([1, Lq], F32, name="scr2")
        nc.scalar.activation(out=scr2, in_=ones_f, func=exp_f)
        for bb in range(1, BPC):
            nc.scalar.dma_start(out=q_tiles[bb], in_=q_ap[bb])

        for bb in range(1, BPC):
            gate(c_tiles[bb], c_tiles[bb - 1], (0,))
            nc.sync.dma_start(out=c_tiles[bb], in_=c_src(bb))

        S = [dict() for _ in range(BPC)]  # per-batch tile state

        def stage_A(b):
            """c transposes (f32r) -> qs^T -> sT matmuls + s0 aug -> exp"""
            c_sb = c_tiles[b]
            q_sb = q_tiles[b]

            # c -> bf16 once per j-chunk (spread across engines); bf16
            # transposes then run at 1 cy/row with half the PSUM traffic
            c_bf = cbfpool.tile([128, 4, H], BF16, name="c_bf")
            nc.vector.tensor_copy(out=c_bf[:, 0, :], in_=c_sb[:, 0, :])
            nc.scalar.activation(out=c_bf[:, 1, :], in_=c_sb[:, 1, :], func=copy_f)
            nc.gpsimd.tensor_copy(out=c_bf[:, 2, :], in_=c_sb[:, 2, :])
            nc.vector.tensor_copy(out=c_bf[:, 3, :], in_=c_sb[:, 3, :])

            cT = ctpool.tile([128, 4, H], BF16, name="cT")
            for j in range(4):
                pt_c = ps_tr.tile([128, 4, 128], BF16, name="pt_c", tag="tr")
                for f in range(4):
                    nc.tensor.transpose(pt_c[:, f, :], c_bf[:, j, ts(f, 128)], ident)
                if j % 2 == 0:
                    nc.vector.tensor_copy(out=cT[:, :, ts(j, 128)], in_=pt_c)
                else:
                    nc.scalar.activation(
                        out=cT[:, :, ts(j, 128)], in_=pt_c, func=copy_f
                    )

            # qs^T = (q * cqw)^T via PE transpose + per-partition cqw scale
            lhsT = lhpool.tile([128, 4, Lq], BF16, name="lhsT")
            pt_q = ps_trq.tile([128, 4, Lq], F32R, name="pt_q", tag="trq")
            for f in range(4):
                nc.tensor.transpose(pt_q[:, f, :], q_sb[:, ts(f, 128)], identf[0:Lq, 0:Lq])
            for f in range(4):
                nc.vector.tensor_scalar_mul(
                    lhsT[:, f, :], pt_q[:, f, :], cqw_t[:, f : f + 1]
                )
            # bf16 q for stage C's a-matmul rhs; Pool is idle and the result
            # is not needed until C(b), so its slow cast rate is fine
            q_bf = qbpool.tile([Lq, H], BF16, name="q_bf")
            nc.gpsimd.tensor_copy(out=q_bf, in_=q_sb)

            # sT rows 0..63 = qs @ cT; then s0 broadcast via K=1 f32r matmul
            ps_sT = ps_mm.tile([128, 512], F32, name="ps_sT", tag="big1")
            for f in range(4):
                nc.tensor.matmul(
                    ps_sT[0:Lq, :], lhsT[:, f, :], cT[:, f, :],
                    start=(f == 0), stop=False,
                )
            nc.tensor.matmul(
                ps_sT[0:Lq, :], onesK, s0_t[0:1, ts(b, Lc)],
                start=False, stop=True,
            )

            # E = exp(sT + s1b) in bf16; rowsum (f32) for a2
            E_sb = epool.tile([Lq, H], BF16, name="E_sb")
            rowsum = spool.tile([Lq, 1], F32, name="rowsum")
            nc.scalar.activation(
                out=E_sb, in_=ps_sT[0:Lq, :], func=exp_f,
                bias=s1b_t[:, b : b + 1], scale=1.0, accum_out=rowsum,
            )
            S[b].update(c_bf=c_bf, q_bf=q_bf, E_sb=E_sb, rowsum=rowsum)

        def stage_B(b):
            """normalizers -> E transpose -> M2 = a2^T @ c (bf16)"""
            c_bf2 = S[b]["c_bf"]
            E_sb = S[b]["E_sb"]
            ra2 = spool.tile([Lq, 1], F32, name="ra2")
            nc.vector.reciprocal(ra2, S[b]["rowsum"])

            # column sums of E (normalizer of a1), one batched reciprocal
            ps_S = ps_ab.tile([128, 4], F32, name="ps_S", tag="big2")
            for m in range(4):
                nc.tensor.matmul(
                    ps_S[:, m : m + 1], E_sb[:, ts(m, 128)], ones_col,
                    start=True, stop=True,
                )
            rS = spool.tile([128, 4], F32, name="rS")
            nc.vector.reciprocal(rS, ps_S)

            # E^T chunks for M2's lhsT (f32r to match the f32 c rhs)
            pt_a = ps_trq.tile([128, 4, Lq], BF16, name="pt_a", tag="trq")
            for f in range(4):
                nc.tensor.transpose(pt_a[:, f, :], E_sb[:, ts(f, 128)], ident[0:Lq, 0:Lq])
            a2n = btpool.tile([128, 4, Lq], BF16, name="a2n")
            nc.vector.tensor_copy(out=a2n, in_=pt_a)

            # M2 = a2^T @ c  [Lq, H]; evac applies ra2, casts to bf16
            ps_M2 = ps_mm.tile([128, 512], F32, name="ps_M2", tag="big1")
            for j in range(4):
                nc.tensor.matmul(
                    ps_M2[0:Lq, :], a2n[:, j, :], c_bf2[:, j, :],
                    start=(j == 0), stop=(j == 3),
                )
            M2_sb = mpool.tile([Lq, H], BF16, name="M2_sb")
            nc.scalar.activation(
                out=M2_sb, in_=ps_M2[0:Lq, :], func=copy_f, scale=ra2
            )
            S[b].update(rS=rS, M2_sb=M2_sb)

        def stage_C(b, ms=(0, 1, 2, 3)):
            """per i-tile: a = a1@q, b = a1@M2, rS-scaled bf16 evacs"""
            q_bf = S[b]["q_bf"]
            E_sb = S[b]["E_sb"]
            rS = S[b]["rS"]
            M2_sb = S[b]["M2_sb"]
            if "stage" not in S[b]:
                S[b]["stage"] = opool.tile([128, 4, 2, H], BF16, name="stage")
            stage = S[b]["stage"]
            for m in ms:
                ps_a = ps_ab.tile([128, 512], F32, name="ps_a", tag="big2")
                nc.tensor.matmul(
                    ps_a, E_sb[:, ts(m, 128)], q_bf, start=True, stop=True
                )
                nc.scalar.activation(
                    out=stage[:, m, 0, :], in_=ps_a, func=copy_f,
                    scale=rS[:, m : m + 1],
                )
                ps_b = ps_ab.tile([128, 512], F32, name="ps_b", tag="big2")
                nc.tensor.matmul(
                    ps_b, E_sb[:, ts(m, 128)], M2_sb, start=True, stop=True
                )
                nc.vector.tensor_scalar_mul(
                    stage[:, m, 1, :], ps_b, rS[:, m : m + 1]
                )
            # half-batch stores start HBM writes as soon as two i-tiles are
            # done instead of waiting for the full batch
            o_view = out_ap[b]
            if ms[-1] == 1:
                nc.sync.dma_start(out=o_view[:, 0:2], in_=stage[:, 0:2])
            elif ms[-1] == 3:
                if ms[0] == 0:
                    nc.sync.dma_start(out=o_view[:, 0:2], in_=stage[:, 0:2])
                nc.sync.dma_start(out=o_view[:, 2:4], in_=stage[:, 2:4])

        # emission order: A stages early (deps land early), C split in halves
        # to interleave with B so every engine queue always has ready work
        stage_A(0)
        stage_A(1)
        stage_B(0)
        stage_A(2)
        stage_B(1)
        stage_C(0, (0, 1))
        stage_A(3)
        stage_B(2)
        stage_C(0, (2, 3))
        stage_C(1, (0, 1))
        stage_B(3)
        stage_C(1, (2, 3))
        stage_C(2, (0, 1))
        stage_C(2, (2, 3))
        stage_C(3)

    nc.compile()
    return nc


def _numpy_fallback(c, q, c_mask, q_mask, c_weight, q_weight, cq_weight, bias):
    NEG_INF = -1e30
    s0 = c @ c_weight
    s1 = (q @ q_weight).transpose(0, 2, 1)
    s2 = np.einsum("bih,bjh->bij", c * cq_weight, q)
    s = s0 + s1 + s2 + bias

    def softmax(x, mask, axis):
        logits = np.where(mask, x, NEG_INF)
        m = logits.max(axis=axis, keepdims=True)
        e = np.exp(logits - m)
        return e / e.sum(axis=axis, keepdims=True)

    a1 = softmax(s, q_mask[:, None, :], 2)
    a2 = softmax(s, c_mask[:, :, None], 1)
    a = np.einsum("bij,bjh->bih", a1, q)
    bb = np.einsum("bij,bjk->bik", np.einsum("bik,bjk->bij", a1, a2), c)
    return np.concatenate([c, a, c * a, c * bb], axis=2).astype(np.float32)


def _make_in_maps(c, q, cq_weight, c_weight, q_weight, bias):
    cqw = np.ascontiguousarray(np.asarray(cq_weight, np.float32).reshape(H))
    cwgt = np.asarray(c_weight, np.float32).reshape(H)
    qwgt = np.asarray(q_weight, np.float32).reshape(H)
    b0 = float(np.asarray(bias, np.float32).reshape(1)[0])
    s0 = (c.reshape(-1, H) @ cwgt).reshape(B, Lc).astype(np.float32)
    # device cT columns are in kappa = j*128 + p order where row i = 4p + j
    s0 = np.ascontiguousarray(
        s0.reshape(B, 128, 4).transpose(0, 2, 1).reshape(B, Lc)
    )
    s1b = ((q.reshape(-1, H) @ qwgt).reshape(B, Lq) + b0).astype(np.float32)
    in_maps = []
    for k in range(N_CORES):
        sl = slice(k * BPC, (k + 1) * BPC)
        in_maps.append(
            {
                "c": np.ascontiguousarray(c[sl]),
                "q": np.ascontiguousarray(q[sl]),
                "cqw": cqw,
                "s0": np.ascontiguousarray(s0[sl]),
                "s1b": np.ascontiguousarray(s1b[sl]),
            }
        )
    return in_maps


def _assemble(c, results):
    out = np.empty((B, Lc, 4 * H), dtype=np.float32)
    out[:, :, 0:H] = c
    for k in range(N_CORES):
        sl = slice(k * BPC, (k + 1) * BPC)
        ab = results[k]["out"].reshape(BPC, Lc, 2, H).astype(np.float32)
        a = ab[:, :, 0, :]
        bb = ab[:, :, 1, :]
        ck = c[sl]
        out[sl, :, H : 2 * H] = a
        np.multiply(ck, a, out=out[sl, :, 2 * H : 3 * H])
        np.multiply(ck, bb, out=out[sl, :, 3 * H : 4 * H])
    return out


def kernel(c, q, c_mask, q_mask, c_weight, q_weight, cq_weight, bias, **_):
    c = np.asarray(c, dtype=np.float32)
    q = np.asarray(q, dtype=np.float32)
    if not (np.all(c_mask) and np.all(q_mask)):
        # masks are all-ones per the problem spec; keep a correct fallback
        return _numpy_fallback(
            c, q, np.asarray(c_mask), np.asarray(q_mask),
            np.asarray(c_weight, np.float32), np.asarray(q_weight, np.float32),
            np.asarray(cq_weight, np.float32), np.asarray(bias, np.float32),
        )

    if "nc" not in _CACHE:
        _CACHE["nc"] = _build_program()
    nc = _CACHE["nc"]

    in_maps = _make_in_maps(c, q, cq_weight, c_weight, q_weight, bias)
    res = run_bass_kernel_spmd(nc, in_maps, core_ids=list(range(N_CORES)))
    return _assemble(c, res.results)


# revision 24
# speedup vs baseline: 1.1965x; 1.0114x over previous
"""BiDAF attention kernel for 8 Trainium2 NeuronCores.

Data-parallel over batch (B=32 -> 4 per core). Per batch, on-chip:
  sT[j,i] = (q*cqw) @ c^T + s0[i] + (s1[j]+bias)   (bf16 matmuls, fp32 accum)
  s0 = c @ c_weight and s1b = q @ q_weight + bias are host-precomputed
  (tiny rank-1 terms); s0 enters via a K=1 f32r matmul accumulated into the
  same PSUM bank, s1b via the exp activation's per-partition bias.
  E = exp(sT)  (one exp serves both softmaxes; rowsum via accum_out)
  a1 normalization deferred: rS=1/colsum(E) scales the a/b PSUM evacuations;
  a2 normalization deferred: ra2=1/rowsum(E) folds into the M2 evacuation.
  a = a1 @ q; b = a1 @ (a2^T @ c); device stores [a, b] in bf16.
Key perf structure vs the previous version:
  - c is never cast on an engine: PE transposes c as f32r (1.5 cy/row) and
    the PSUM->SBUF evacuation casts to bf16 for the sT matmul rhs.
  - q is cast f32->bf16 in flight by the gpsimd software-DGE DMA.
  - M2 = a2^T @ c runs as an f32r matmul straight off the f32 c tile.
  - c*a and c*b are computed on the host during unshard (the device writes
    only [a, b]: 4 MiB of bf16 HBM writes per core instead of 6+).
  - identity + small consts issue ahead of the bulk loads; one batched 1 MiB
    store per batch; stages interleaved so the PE pipeline never drains
    (p-state ramp: a continuously-busy PE runs 2x faster than one with gaps).
"""

import sys

if "/opt/trn_rl_repo" not in sys.path:
    sys.path.insert(0, "/opt/trn_rl_repo")

from contextlib import ExitStack

import numpy as np

import concourse.bacc as bacc
import concourse.bass as bass
import concourse.mybir as mybir
from concourse.bass import ts
from concourse.bass_utils import run_bass_kernel_spmd
from concourse.masks import make_identity
from concourse.tile import TileContext

N_CORES = 8
B, Lc, Lq, H = 32, 512, 64, 512
BPC = B // N_CORES  # batches per core
F32 = mybir.dt.float32
F32R = mybir.dt.float32r
BF16 = mybir.dt.bfloat16

_CACHE = {}


def _build_program():
    nc = bacc.Bacc("TRN2", target_bir_lowering=False, debug=False, num_devices=N_CORES)
    c_h = nc.dram_tensor("c", [BPC, Lc, H], F32R, kind="ExternalInput")
    q_h = nc.dram_tensor("q", [BPC, Lq, H], F32R, kind="ExternalInput")
    cqw_h = nc.dram_tensor("cqw", [H], F32, kind="ExternalInput")
    s0_h = nc.dram_tensor("s0", [BPC, Lc], F32R, kind="ExternalInput")
    s1b_h = nc.dram_tensor("s1b", [BPC, Lq], F32, kind="ExternalInput")
    out_h = nc.dram_tensor("out", [BPC, 128, 4, 2, H], BF16, kind="ExternalOutput")

    c_ap = c_h.ap()
    q_ap = q_h.ap()
    out_ap = out_h.ap()

    exp_f = mybir.ActivationFunctionType.Exp
    copy_f = mybir.ActivationFunctionType.Copy

    with TileContext(nc) as tc, ExitStack() as ctx:
        const = ctx.enter_context(tc.tile_pool(name="const", bufs=1))
        cpool = ctx.enter_context(tc.tile_pool(name="cpool", bufs=4))
        ctpool = ctx.enter_context(tc.tile_pool(name="ctpool", bufs=2))
        cbfpool = ctx.enter_context(tc.tile_pool(name="cbfpool", bufs=4))
        lhpool = ctx.enter_context(tc.tile_pool(name="lhpool", bufs=2))
        qpool = ctx.enter_context(tc.tile_pool(name="qpool", bufs=4))
        qbpool = ctx.enter_context(tc.tile_pool(name="qbpool", bufs=4))
        spool = ctx.enter_context(tc.tile_pool(name="spool", bufs=12))
        epool = ctx.enter_context(tc.tile_pool(name="epool", bufs=4))
        btpool = ctx.enter_context(tc.tile_pool(name="btpool", bufs=2))
        mpool = ctx.enter_context(tc.tile_pool(name="mpool", bufs=3))
        opool = ctx.enter_context(tc.tile_pool(name="opool", bufs=2))
        ps_tr = ctx.enter_context(tc.tile_pool(name="ps_tr", bufs=2, space="PSUM"))
        ps_trq = ctx.enter_context(tc.tile_pool(name="ps_trq", bufs=1, space="PSUM"))
        ps_mm = ctx.enter_context(tc.tile_pool(name="ps_mm", bufs=2, space="PSUM"))
        ps_ab = ctx.enter_context(tc.tile_pool(name="ps_ab", bufs=3, space="PSUM"))

        # ---- constants + loads: identity first on the gpsimd queue so the
        # first PE transposes are never gated on it; q casts f32->bf16 in
        # flight (SWDGE); c goes f32 on the sync HWDGE queue; small consts
        # issue from the scalar HWDGE queue ahead of its activation work ----
        ident = const.tile([128, 128], BF16, name="ident")
        make_identity(nc, ident)
        identf = const.tile([128, 128], F32R, name="identf")
        nc.vector.tensor_copy(out=identf, in_=ident)

        ones_col = const.tile([Lq, 1], BF16, name="ones_col")
        nc.vector.memset(ones_col, 1.0)
        ones_f = const.tile([1, Lq], F32, name="ones_f")
        nc.vector.memset(ones_f, 1.0)
        onesK = const.tile([1, Lq], F32R, name="onesK")
        nc.vector.tensor_copy(out=onesK, in_=ones_f)

        # Load scheduling: a single DMA stream tops out at ~200 GB/s and
        # concurrent DMAs split HBM bandwidth evenly, so c0 goes out as 4
        # partition-range quarters, two per HWDGE queue, issued before
        # anything else (full-bandwidth arrival of the first batch by ~10us).
        # c1..c3 are half-DMA pairs on the sync queue, each pair gated on the
        # previous batch's completion via 16-byte gpsimd copies into the next
        # tile's head bytes (the DMA waits on the WAW dep; one gate per
        # previous-writer partition range so every sub-DMA is covered).
        q_tiles = {}
        c_tiles = {}
        for bb in range(BPC):
            q_tiles[bb] = qpool.tile([Lq, H], F32R, name="q_sb")
            c_tiles[bb] = cpool.tile([128, 4, H], F32R, name="c_sb")

        def c_src(bb):
            return c_ap[bb].rearrange("(p j) h -> p j h", p=128)

        def _head(t, p):
            return t[p : p + 1, 0, 0:4] if len(t.shape) == 3 else t[p : p + 1, 0:4]

        def gate(nxt, prev, parts):
            for p in parts:
                nc.gpsimd.tensor_copy(out=_head(nxt, p), in_=_head(prev, p))

        nc.sync.dma_start(out=c_tiles[0], in_=c_src(0))

        nc.scalar.dma_start(out=q_tiles[0], in_=q_ap[0])
        cqw_t = const.tile([128, 4], F32, name="cqw_t")
        nc.scalar.dma_start(
            out=cqw_t, in_=bass.AP(tensor=cqw_h, offset=0, ap=[[1, 128], [128, 4]])
        )
        s1b_t = const.tile([Lq, BPC], F32, name="s1b_t")
        nc.scalar.dma_start(
            out=s1b_t, in_=bass.AP(tensor=s1b_h, offset=0, ap=[[1, Lq], [Lq, BPC]])
        )
        s0_t = const.tile([1, BPC * Lc], F32R, name="s0_t")
        nc.scalar.dma_start(
            out=s0_t, in_=bass.AP(tensor=s0_h, offset=0, ap=[[1, 1], [1, BPC * Lc]])
        )
        # prime the activation table (1.3us) while the loads are in flight
        scr2 = const.tile([1, Lq], F32, name="scr2")
        nc.scalar.activation(out=scr2, in_=ones_f, func=exp_f)
        gate(q_tiles[1], c_tiles[0], (0,))
        nc.scalar.dma_start(out=q_tiles[1], in_=q_ap[1])
        gate(q_tiles[2], q_tiles[1], (0,))
        nc.scalar.dma_start(out=q_tiles[2], in_=q_ap[2])
        gate(q_tiles[3], q_tiles[2], (0,))
        nc.scalar.dma_start(out=q_tiles[3], in_=q_ap[3])

        for bb in range(1, BPC):
            gate(c_tiles[bb], c_tiles[bb - 1], (0,))
            nc.sync.dma_start(out=c_tiles[bb], in_=c_src(bb))

        # PE clock warmup: the tensor engine ramps to full clock only after
        # ~3us of continuous work. A string of tiny transposes keeps it busy
        # from the moment the identity exists until c0 lands, so the first
        # real transposes run at full speed instead of ramping from idle.
        warm = ps_ab.tile([16, 16], BF16, name="warm", tag="big2")
        for _ in range(56):
            nc.tensor.transpose(warm, ident[:, 0:16], ident[:, 0:16])

        S = [dict() for _ in range(BPC)]  # per-batch tile state

        def stage_A(b):
            """c transposes (f32r) -> qs^T -> sT matmuls + s0 aug -> exp"""
            c_sb = c_tiles[b]
            q_sb = q_tiles[b]

            # c -> bf16 once per j-chunk (spread across engines); bf16
            # transposes then run at 1 cy/row with half the PSUM traffic
            c_bf = cbfpool.tile([128, 4, H], BF16, name="c_bf")
            nc.vector.tensor_copy(out=c_bf[:, 0, :], in_=c_sb[:, 0, :])
            nc.scalar.activation(out=c_bf[:, 1, :], in_=c_sb[:, 1, :], func=copy_f)
            nc.gpsimd.tensor_copy(out=c_bf[:, 2, :], in_=c_sb[:, 2, :])
            nc.vector.tensor_copy(out=c_bf[:, 3, :], in_=c_sb[:, 3, :])

            cT = ctpool.tile([128, 4, H], BF16, name="cT")
            for j in range(4):
                pt_c = ps_tr.tile([128, 4, 128], BF16, name="pt_c", tag="tr")
                for f in range(4):
                    nc.tensor.transpose(pt_c[:, f, :], c_bf[:, j, ts(f, 128)], ident)
                if j % 2 == 0:
                    nc.vector.tensor_copy(out=cT[:, :, ts(j, 128)], in_=pt_c)
                else:
                    nc.scalar.activation(
                        out=cT[:, :, ts(j, 128)], in_=pt_c, func=copy_f
                    )

            # qs^T = (q * cqw)^T via PE transpose + per-partition cqw scale
            lhsT = lhpool.tile([128, 4, Lq], BF16, name="lhsT")
            pt_q = ps_trq.tile([128, 4, Lq], F32R, name="pt_q", tag="trq")
            for f in range(4):
                nc.tensor.transpose(pt_q[:, f, :], q_sb[:, ts(f, 128)], identf[0:Lq, 0:Lq])
            for f in range(4):
                nc.vector.tensor_scalar_mul(
                    lhsT[:, f, :], pt_q[:, f, :], cqw_t[:, f : f + 1]
                )
            # bf16 q for stage C's a-matmul rhs; Pool is idle and the result
            # is not needed until C(b), so its slow cast rate is fine
            q_bf = qbpool.tile([Lq, H], BF16, name="q_bf")
            nc.gpsimd.tensor_copy(out=q_bf, in_=q_sb)

            # sT rows 0..63 = qs @ cT; then s0 broadcast via K=1 f32r matmul
            ps_sT = ps_mm.tile([128, 512], F32, name="ps_sT", tag="big1")
            for f in range(4):
                nc.tensor.matmul(
                    ps_sT[0:Lq, :], lhsT[:, f, :], cT[:, f, :],
                    start=(f == 0), stop=False,
                )
            nc.tensor.matmul(
                ps_sT[0:Lq, :], onesK, s0_t[0:1, ts(b, Lc)],
                start=False, stop=True,
            )

            # E = exp(sT + s1b) in bf16; rowsum (f32) for a2
            E_sb = epool.tile([Lq, H], BF16, name="E_sb")
            rowsum = spool.tile([Lq, 1], F32, name="rowsum")
            nc.scalar.activation(
                out=E_sb, in_=ps_sT[0:Lq, :], func=exp_f,
                bias=s1b_t[:, b : b + 1], scale=1.0, accum_out=rowsum,
            )
            S[b].update(c_bf=c_bf, q_bf=q_bf, E_sb=E_sb, rowsum=rowsum)

        def stage_B(b):
            """normalizers -> E transpose -> M2 = a2^T @ c (bf16)"""
            c_bf2 = S[b]["c_bf"]
            E_sb = S[b]["E_sb"]
            ra2 = spool.tile([Lq, 1], F32, name="ra2")
            nc.vector.reciprocal(ra2, S[b]["rowsum"])

            # column sums of E (normalizer of a1), one batched reciprocal
            ps_S = ps_ab.tile([128, 4], F32, name="ps_S", tag="big2")
            for m in range(4):
                nc.tensor.matmul(
                    ps_S[:, m : m + 1], E_sb[:, ts(m, 128)], ones_col,
                    start=True, stop=True,
                )
            rS = spool.tile([128, 4], F32, name="rS")
            nc.vector.reciprocal(rS, ps_S)

            # E^T chunks for M2's lhsT (f32r to match the f32 c rhs)
            pt_a = ps_trq.tile([128, 4, Lq], BF16, name="pt_a", tag="trq")
            for f in range(4):
                nc.tensor.transpose(pt_a[:, f, :], E_sb[:, ts(f, 128)], ident[0:Lq, 0:Lq])
            a2n = btpool.tile([128, 4, Lq], BF16, name="a2n")
            nc.vector.tensor_copy(out=a2n, in_=pt_a)

            # M2 = a2^T @ c  [Lq, H]; evac applies ra2, casts to bf16
            ps_M2 = ps_mm.tile([128, 512], F32, name="ps_M2", tag="big1")
            for j in range(4):
                nc.tensor.matmul(
                    ps_M2[0:Lq, :], a2n[:, j, :], c_bf2[:, j, :],
                    start=(j == 0), stop=(j == 3),
                )
            M2_sb = mpool.tile([Lq, H], BF16, name="M2_sb")
            nc.scalar.activation(
                out=M2_sb, in_=ps_M2[0:Lq, :], func=copy_f, scale=ra2
            )
            S[b].update(rS=rS, M2_sb=M2_sb)

        def stage_C(b, ms=(0, 1, 2, 3)):
            """per i-tile: a = a1@q, b = a1@M2, rS-scaled bf16 evacs"""
            q_bf = S[b]["q_bf"]
            E_sb = S[b]["E_sb"]
            rS = S[b]["rS"]
            M2_sb = S[b]["M2_sb"]
            if "stage" not in S[b]:
                S[b]["stage"] = opool.tile([128, 4, 2, H], BF16, name="stage")
            stage = S[b]["stage"]
            for m in ms:
                ps_a = ps_ab.tile([128, 512], F32, name="ps_a", tag="big2")
                nc.tensor.matmul(
                    ps_a, E_sb[:, ts(m, 128)], q_bf, start=True, stop=True
                )
                nc.scalar.activation(
                    out=stage[:, m, 0, :], in_=ps_a, func=copy_f,
                    scale=rS[:, m : m + 1],
                )
                ps_b = ps_ab.tile([128, 512], F32, name="ps_b", tag="big2")
                nc.tensor.matmul(
                    ps_b, E_sb[:, ts(m, 128)], M2_sb, start=True, stop=True
                )
                nc.vector.tensor_scalar_mul(
                    stage[:, m, 1, :], ps_b, rS[:, m : m + 1]
                )
            # half-batch stores start HBM writes as soon as two i-tiles are
            # done instead of waiting for the full batch
            o_view = out_ap[b]
            if ms[-1] == 1:
                nc.sync.dma_start(out=o_view[:, 0:2], in_=stage[:, 0:2])
            elif ms[-1] == 3:
                if ms[0] == 0:
                    nc.sync.dma_start(out=o_view[:, 0:2], in_=stage[:, 0:2])
                nc.sync.dma_start(out=o_view[:, 2:4], in_=stage[:, 2:4])

        # emission order: A stages early (deps land early), C split in halves
        # to interleave with B so every engine queue always has ready work
        stage_A(0)
        stage_A(1)
        stage_B(0)
        stage_A(2)
        stage_B(1)
        stage_C(0, (0, 1))
        stage_A(3)
        stage_B(2)
        stage_C(0, (2, 3))
        stage_C(1, (0, 1))
        stage_B(3)
        stage_C(1, (2, 3))
        stage_C(2, (0, 1))
        stage_C(2, (2, 3))
        stage_C(3)

    nc.compile()
    return nc


def _numpy_fallback(c, q, c_mask, q_mask, c_weight, q_weight, cq_weight, bias):
    NEG_INF = -1e30
    s0 = c @ c_weight
    s1 = (q @ q_weight).transpose(0, 2, 1)
    s2 = np.einsum("bih,bjh->bij", c * cq_weight, q)
    s = s0 + s1 + s2 + bias

    def softmax(x, mask, axis):
        logits = np.where(mask, x, NEG_INF)
        m = logits.max(axis=axis, keepdims=True)
        e = np.exp(logits - m)
        return e / e.sum(axis=axis, keepdims=True)

    a1 = softmax(s, q_mask[:, None, :], 2)
    a2 = softmax(s, c_mask[:, :, None], 1)
    a = np.einsum("bij,bjh->bih", a1, q)
    bb = np.einsum("bij,bjk->bik", np.einsum("bik,bjk->bij", a1, a2), c)
    return np.concatenate([c, a, c * a, c * bb], axis=2).astype(np.float32)


def _make_in_maps(c, q, cq_weight, c_weight, q_weight, bias):
    cqw = np.ascontiguousarray(np.asarray(cq_weight, np.float32).reshape(H))
    cwgt = np.asarray(c_weight, np.float32).reshape(H)
    qwgt = np.asarray(q_weight, np.float32).reshape(H)
    b0 = float(np.asarray(bias, np.float32).reshape(1)[0])
    s0 = (c.reshape(-1, H) @ cwgt).reshape(B, Lc).astype(np.float32)
    # device cT columns are in kappa = j*128 + p order where row i = 4p + j
    s0 = np.ascontiguousarray(
        s0.reshape(B, 128, 4).transpose(0, 2, 1).reshape(B, Lc)
    )
    s1b = ((q.reshape(-1, H) @ qwgt).reshape(B, Lq) + b0).astype(np.float32)
    in_maps = []
    for k in range(N_CORES):
        sl = slice(k * BPC, (k + 1) * BPC)
        in_maps.append(
            {
                "c": np.ascontiguousarray(c[sl]),
                "q": np.ascontiguousarray(q[sl]),
                "cqw": cqw,
                "s0": np.ascontiguousarray(s0[sl]),
                "s1b": np.ascontiguousarray(s1b[sl]),
            }
        )
    return in_maps


def _assemble(c, results):
    out = np.empty((B, Lc, 4 * H), dtype=np.float32)
    out[:, :, 0:H] = c
    for k in range(N_CORES):
        sl = slice(k * BPC, (k + 1) * BPC)
        ab = results[k]["out"].reshape(BPC, Lc, 2, H).astype(np.float32)
        a = ab[:, :, 0, :]
        bb = ab[:, :, 1, :]
        ck = c[sl]
        out[sl, :, H : 2 * H] = a
        np.multiply(ck, a, out=out[sl, :, 2 * H : 3 * H])
        np.multiply(ck, bb, out=out[sl, :, 3 * H : 4 * H])
    return out


def kernel(c, q, c_mask, q_mask, c_weight, q_weight, cq_weight, bias, **_):
    c = np.asarray(c, dtype=np.float32)
    q = np.asarray(q, dtype=np.float32)
    if not (np.all(c_mask) and np.all(q_mask)):
        # masks are all-ones per the problem spec; keep a correct fallback
        return _numpy_fallback(
            c, q, np.asarray(c_mask), np.asarray(q_mask),
            np.asarray(c_weight, np.float32), np.asarray(q_weight, np.float32),
            np.asarray(cq_weight, np.float32), np.asarray(bias, np.float32),
        )

    if "nc" not in _CACHE:
        _CACHE["nc"] = _build_program()
    nc = _CACHE["nc"]

    in_maps = _make_in_maps(c, q, cq_weight, c_weight, q_weight, bias)
    res = run_bass_kernel_spmd(nc, in_maps, core_ids=list(range(N_CORES)))
    return _assemble(c, res.results)
